# revision 48
# baseline (speedup 1.0000x reference)
"""Multi-head attention (B=2, S=2048, D=1024, H=16, HD=64) on 8 trn2 cores.

Sharding: core c -> (batch b = c//4, head-group hg = c%4, heads 4*hg..4*hg+3).
Each core computes its 4 heads' attention for its batch plus the partial
output projection (ctx @ Wo_slice); the host sums the 4 partials per batch
and adds bo.  setup_inputs() guarantees inputs_kv is inputs_q, which the
host verifies (falling back to a numpy reference otherwise); the host also
pre-transposes x once per batch and casts it (and Wq/Wk/Wv) to bf16, so the
device program needs no transposes at all.

The kernel is built around the observation that the softmax exp is the
hard floor: 16.8M elements/core through the ACT spline at ~1 elem/lane/
cycle @1.2GHz = ~130-147us.  Everything else (projections, score matmuls,
ctx matmuls, out-projection) is arranged so the ACT engine streams exp
continuously from ~14us after kernel start:

  1. x^T arrives in four 512-column chunks; q^T/k^T projections for head
     pair 0 chase the chunks (PE, bf16, full-rate), so scores for (qc0,
     pair0) issue per 4-kc batch as soon as each chunk is projected.
  2. scores: per kc a row-packed pair of K=64 bf16 matmuls (heads in
     disjoint PE row groups run concurrently), psum [128, 2*512].
  3. exp via ACT reading psum, scale=1/sqrt(HD) folded in, bf16 out into a
     24-deep rotating pool of per-kc tiles (decouples ACT from the ctx
     consumers by more than one full sweep).
  4. ctx^T per head (PE): lhsT = ve[:, kc, :] where ve packs v_h into the
     head's own 64 columns plus a ones-column, so one psum row is the
     softmax denominator for free.
  5. normalize: denominator rows -> sel-matmul broadcast ->
     reciprocal_approx_fast -> per-half multiply -> ctxn (fp32r).
  6. out-projection (PE, fp32r) accumulated over the two pairs, DMA'd out
     per [128, 512] chunk as produced.

A short burst of dummy sel-matmuls at t~6-10us keeps the PE HAM busy so
the real projections start at the warm 2.4GHz clock.
"""

import os
from contextlib import ExitStack

import ml_dtypes
import numpy as np

import concourse.mybir as mybir
import concourse.tile as tile
from concourse import bacc
from concourse.bass_utils import run_bass_kernel_spmd

FP32 = mybir.dt.float32
FP32R = mybir.dt.float32r
BF16 = mybir.dt.bfloat16
AF = mybir.ActivationFunctionType

B, S, D, H, HD = 2, 2048, 1024, 16, 64
NCORES = 8
HPC = 4  # heads per core
PAIRS = 2  # head pairs per core
DC = D // 128  # 8 contraction chunks
SC = 4  # s-chunks for the pipelined load/projection
SCW = S // SC  # 512
KC = S // 128  # 16 k chunks
QC = 4  # q chunks of 512
QW = 512
SCALE = 1.0 / np.sqrt(HD)
EXP_BUFS = 30  # rotating per-kc exp tiles (> ctx lag + margin)
CTX_LAG = 20  # ctx trails scores by this many ticks (spreads the v-proj)
WARMUP_MMS = 16

_PROG_CACHE = {}
LAST_EXEC_NS = None


def _build_program(zero_bias):
    nc = bacc.Bacc(None, target_bir_lowering=False, debug=False)

    # all bulk inputs are pre-swizzled on the host so every DMA below reads
    # large contiguous spans per partition (a single dma_start tops out at
    # ~70 GB/s; strided small-element reads are far worse)
    xt = nc.declare_dram_parameter("xt", [128, SC * DC * SCW], BF16, isOutput=False)
    wq = nc.declare_dram_parameter("wq", [128, DC * 256], BF16, isOutput=False)
    wk = nc.declare_dram_parameter("wk", [128, DC * 256], BF16, isOutput=False)
    wv = nc.declare_dram_parameter("wv", [128, DC * 256], BF16, isOutput=False)
    wo = nc.declare_dram_parameter("wo", [128, 2 * D], FP32R, isOutput=False)
    bq = nc.declare_dram_parameter("bq", [128, 2], FP32, isOutput=False)
    bk = nc.declare_dram_parameter("bk", [128, 2], FP32, isOutput=False)
    bv = nc.declare_dram_parameter("bv", [1, 256], FP32, isOutput=False)
    out_p = nc.declare_dram_parameter("out_p", [S, D], FP32, isOutput=True)

    # sel[k, m] broadcasts r2 row 64 to output rows 0-63 and row 0 to
    # output rows 64-127 (the two heads' denominator rows)
    sel_np = np.zeros((128, 128), np.float32)
    sel_np[64, :64] = 1.0
    sel_np[0, 64:] = 1.0
    sel_c = nc.inline_tensor(sel_np, name="sel_c")

    with ExitStack() as ctx:
        tc = ctx.enter_context(tile.TileContext(nc))

        main = ctx.enter_context(tc.tile_pool(name="main", bufs=1))
        attn = ctx.enter_context(tc.tile_pool(name="attn", bufs=1))
        psum = ctx.enter_context(tc.tile_pool(name="psum", bufs=1, space="PSUM"))

        sel = main.tile([128, 128], FP32R)
        r2 = main.tile([128, QW], FP32R)
        zsrc = main.tile([128, QW], FP32)
        bq_sb = main.tile([128, 2], FP32)
        bk_sb = main.tile([128, 2], FP32)
        bv_sb = main.tile([128, 256], FP32)
        wo_sb = main.tile([128, 2, D], FP32R)
        wq_sb = main.tile([128, 2, DC, 128], BF16)
        wk_sb = main.tile([128, 2, DC, 128], BF16)
        wv_sb = main.tile([128, DC, 256], BF16)
        xT = main.tile([128, DC, S], BF16)
        qT2 = [main.tile([128, S], BF16, name=f"qT2_{p}") for p in range(PAIRS)]
        kT2 = [main.tile([128, S], BF16, name=f"kT2_{p}") for p in range(PAIRS)]
        # ve[pair*2+hh]: per-head v with a ones column riding along so the
        # ctx matmul also produces the softmax denominator row:
        #   hh=0: cols 0-63 = v_h0, col 64 = ones, rest junk
        #   hh=1: cols 64-127 = v_h1, col 0 = ones, rest junk
        ve = [
            main.tile([128, KC, 128], BF16, name=f"ve_{i}") for i in range(2 * PAIRS)
        ]
        ones16 = main.tile([128, KC], FP32)
        warm = main.tile([128, 128], BF16)

        # ---- DMA issue: sel first (warmup needs it), weights on the
        # scalar queue (idle until the first exp), x chunks on sync ----
        # A single dma_start tops out at ~70 GB/s and each issue queue runs
        # ~2 transfers concurrently, so the first-wave tensors (x chunk 0 +
        # wk) are split in ~256KB pieces across all three queues; later
        # waves chase their compute deadlines.
        wk_v = wk.rearrange("p (h x) -> p h x", h=2)
        wq_v = wq.rearrange("p (h x) -> p h x", h=2)
        wv_v = wv.rearrange("p (h x) -> p h x", h=2)
        wo_v = wo.rearrange("p (h x) -> p h x", h=2)
        xt_q = xt.rearrange("p (c h x) -> p c h x", c=SC, h=4)

        def x_quarter(eng, sc, qi):
            eng.dma_start(
                out=xT[:, qi * 2 : (qi + 1) * 2, sc * SCW : (sc + 1) * SCW],
                in_=xt_q[:, sc, qi, :],
            )

        # bulk transfers ride the two HWDGE queues only (gpsimd SWDGE is
        # slow to generate descriptors -- tiny tensors only).  First wave:
        # x chunk 0 + the pair-0 halves of wk/wq; later x chunks and the
        # remaining weights chase their compute deadlines.
        # per-queue DMA throughput is ~85-90 GB/s regardless of queue depth,
        # so the first wave (x chunk 0 + pair-0 wk/wq) spreads across all
        # three queues: x quarters on sync+scalar, weights on gpsimd
        if not zero_bias:  # bias tiles gate the first projections
            nc.gpsimd.dma_start(out=bk_sb, in_=bk[:, :])
            nc.gpsimd.dma_start(out=bq_sb, in_=bq[:, :])
        nc.gpsimd.dma_start(out=wk_sb[:, 0, :, :], in_=wk_v[:, 0, :])
        nc.gpsimd.dma_start(out=wq_sb[:, 0, :, :], in_=wq_v[:, 0, :])
        nc.gpsimd.dma_start(out=sel, in_=sel_c[:, :])
        bv_bcast = bv[0:1, :].partition_broadcast(128)
        nc.gpsimd.dma_start(out=bv_sb, in_=bv_bcast)
        if zero_bias:  # still loaded (ve adds use them), just not urgent
            nc.gpsimd.dma_start(out=bk_sb, in_=bk[:, :])
            nc.gpsimd.dma_start(out=bq_sb, in_=bq[:, :])
        x_quarter(nc.sync, 0, 0)
        x_quarter(nc.sync, 0, 1)
        x_quarter(nc.scalar, 0, 2)
        x_quarter(nc.scalar, 0, 3)
        for sc in range(1, SC):
            x_quarter(nc.sync, sc, 0)
            x_quarter(nc.sync, sc, 1)
            x_quarter(nc.scalar, sc, 2)
            x_quarter(nc.scalar, sc, 3)
        nc.scalar.dma_start(out=wk_sb[:, 1, :, :], in_=wk_v[:, 1, :])
        nc.scalar.dma_start(out=wq_sb[:, 1, :, :], in_=wq_v[:, 1, :])
        for hi in range(2):
            nc.scalar.dma_start(
                out=wv_sb[:, hi * 4 : (hi + 1) * 4, :], in_=wv_v[:, hi, :]
            )
        for hi in range(2):
            nc.scalar.dma_start(out=wo_sb[:, hi, :], in_=wo_v[:, hi, :])

        # ---- constants ----
        nc.vector.memset(zsrc, 0.0)
        nc.vector.tensor_copy(r2, zsrc)
        nc.vector.memset(ones16, 1.0)
        nc.vector.memset(warm, 1.0)
        for i in range(2 * PAIRS):
            col = 64 if i % 2 == 0 else 0
            nc.vector.tensor_copy(
                ve[i][:, :, col : col + 1],
                ones16.rearrange("p (a o) -> p a o", o=1),
            )

        # ---- PE warmup: dummy bf16 matmuls on a memset tile (no DMA
        # dependency) while the first x chunk is in flight, so the HAM
        # un-throttles before real work ----
        for wi in range(WARMUP_MMS):
            wps = psum.tile([128, 128], FP32, tag="work", bufs=2, name=f"warm{wi}")
            nc.tensor.matmul(wps, warm, warm, start=True, stop=True)

        # ---- projections ----
        def proj_one(nm, w_sb, b_sb, dest, pair, sc):
            pps = psum.tile(
                [128, SCW], FP32, tag="work", bufs=2, name=f"p{nm}{pair}_{sc}"
            )
            for dc in range(DC):
                nc.tensor.matmul(
                    pps,
                    w_sb[:, pair, dc, :],
                    xT[:, dc, sc * SCW : (sc + 1) * SCW],
                    start=(dc == 0),
                    stop=(dc == DC - 1),
                )
            dst = dest[:, sc * SCW : (sc + 1) * SCW]
            if zero_bias and pair == 0 and sc == 0:
                # exp0's critical path: cast on the (still idle) ACT engine
                nc.scalar.copy(dst, pps)
            elif zero_bias:
                nc.vector.tensor_copy(dst, pps)
            else:
                nc.vector.tensor_scalar_add(dst, pps, b_sb[:, pair : pair + 1])

        def proj_k(pair, sc):
            proj_one("k", wk_sb, bk_sb, kT2[pair], pair, sc)

        def proj_q(pair, sc):
            proj_one("q", wq_sb, bq_sb, qT2[pair], pair, sc)

        def proj_v_rc(rc):
            vps = psum.tile([128, 256], FP32, tag="work", bufs=2, name=f"pv{rc}")
            for dc in range(DC):
                nc.tensor.matmul(
                    vps,
                    xT[:, dc, rc * 128 : (rc + 1) * 128],
                    wv_sb[:, dc, :],
                    start=(dc == 0),
                    stop=(dc == DC - 1),
                )
            for pair in range(PAIRS):
                nc.vector.tensor_add(
                    ve[pair * 2][:, rc, 0:64],
                    vps[:, pair * 128 : pair * 128 + 64],
                    bv_sb[:, pair * 128 : pair * 128 + 64],
                )
                nc.vector.tensor_add(
                    ve[pair * 2 + 1][:, rc, 64:128],
                    vps[:, pair * 128 + 64 : pair * 128 + 128],
                    bv_sb[:, pair * 128 + 64 : pair * 128 + 128],
                )

        # ---- attention primitives (one kc "tick" at a time) ----
        TICKS = [
            (qc, pair, kc)
            for qc in range(QC)
            for pair in range(PAIRS)
            for kc in range(KC)
        ]
        etiles = {}

        def score_tick(i):
            qc, pair, kc = TICKS[i]
            sps = psum.tile(
                [128, 1024], FP32, tag="score", bufs=2, name=f"s{qc}_{pair}_{kc}"
            )
            for hh in range(2):
                h_lo = hh * 64
                nc.tensor.matmul(
                    sps[:, hh * QW : (hh + 1) * QW],
                    kT2[pair][h_lo : h_lo + 64, kc * 128 : (kc + 1) * 128],
                    qT2[pair][h_lo : h_lo + 64, qc * QW : (qc + 1) * QW],
                    start=True,
                    stop=True,
                )
            et = attn.tile(
                [128, 1024],
                BF16,
                tag="expT",
                bufs=EXP_BUFS,
                name=f"et{qc}_{pair}_{kc}",
            )
            nc.scalar.activation(et, sps, AF.Exp, scale=float(SCALE))
            etiles[i] = et

        cns = {}
        ctx_tiles = {}
        bg = []  # deadline-checked background PE work: (label, fn, est_cost_us)

        def make_out_item(qc, qsub, ec, cn_pair):
            def fn():
                out_sb = attn.tile(
                    [128, QW], FP32, tag="outsb", bufs=4, name=f"o{qc}_{qsub}_{ec}"
                )
                r0 = qc * QW + qsub * 128
                ops = psum.tile(
                    [128, QW], FP32, tag="work", bufs=2, name=f"op{qc}{qsub}{ec}"
                )
                for pi in range(PAIRS):
                    nc.tensor.matmul(
                        ops,
                        cn_pair[pi][:, qsub * 128 : (qsub + 1) * 128],
                        wo_sb[:, pi, ec * QW : (ec + 1) * QW],
                        start=(pi == 0),
                        stop=(pi == PAIRS - 1),
                    )
                nc.vector.tensor_copy(out_sb, ops)
                nc.sync.dma_start(
                    out=out_p[r0 : r0 + 128, ec * QW : (ec + 1) * QW],
                    in_=out_sb,
                )

            return fn

        out3_acc = {}

        def make_out3_item(pair, qsub, ec, cn):
            # pair-1 items run in the serial tail: alternate psum tags so the
            # mm -> DVE-add -> mm chain isn't throttled by one 2-deep rotation
            # (the ctx tag is free once the last norm has read its tiles)
            tag = "ctx" if (pair == 1 and (qsub + ec) % 2 == 0) else "work"

            def fn():
                ops = psum.tile(
                    [128, QW], FP32, tag=tag, bufs=2, name=f"o3p{pair}{qsub}{ec}"
                )
                nc.tensor.matmul(
                    ops,
                    cn[:, qsub * 128 : (qsub + 1) * 128],
                    wo_sb[:, pair, ec * QW : (ec + 1) * QW],
                    start=True,
                    stop=True,
                )
                if pair == 0:
                    acc = attn.tile(
                        [128, QW], FP32, tag="o3acc", bufs=8, name=f"o3a{qsub}{ec}"
                    )
                    nc.vector.tensor_copy(acc, ops)
                    out3_acc[(qsub, ec)] = acc
                else:
                    acc = out3_acc.pop((qsub, ec))
                    out_sb = attn.tile(
                        [128, QW], FP32, tag="outsb", bufs=4, name=f"o3b{qsub}{ec}"
                    )
                    nc.vector.tensor_add(out_sb, ops, acc)
                    r0 = (QC - 1) * QW + qsub * 128
                    nc.sync.dma_start(
                        out=out_p[r0 : r0 + 128, ec * QW : (ec + 1) * QW],
                        in_=out_sb,
                    )

            return fn

        def norm_and_queue_out(qc, pair, ctxh):
            if qc == QC - 1 and pair == 1:
                # make sure the last q-chunk's pair-0 out-projection ran
                # before the serial tail begins
                for qs in range(4):
                    for e2 in range(2):
                        need(f"o30_{qs}_{e2}")
            # normalize: denominator rows -> r2 -> sel-matmul broadcast ->
            # approx reciprocal -> per-half mul
            nc.vector.tensor_copy(r2[64:65, :], ctxh[0][64:65, :])
            nc.vector.tensor_copy(r2[0:1, :], ctxh[1][0:1, :])
            bps = psum.tile([128, QW], FP32, tag="work", bufs=2, name=f"b{qc}_{pair}")
            nc.tensor.matmul(bps, sel, r2, start=True, stop=True)
            rinv = attn.tile([128, QW], FP32, tag="rinv", bufs=2, name=f"ri{qc}{pair}")
            nc.vector.reciprocal_approx_fast(rinv, bps)
            # the cn pool rotates 4 deep; the slot being reused is still
            # read by the victim q-chunk's out-projection matmuls, which may
            # linger in the background queue BEHIND this point of the PE
            # stream -- force them out first or the PE deadlocks on itself
            a = 2 * qc + pair
            if a >= 4:
                vq = (a - 4) // 2
                if vq < QC - 1:
                    for qs in range(4):
                        for e2 in range(2):
                            need(f"out{vq}_{qs}_{e2}")
            cn = attn.tile([128, QW], FP32R, tag="ctxn", bufs=4, name=f"cn{qc}{pair}")
            nc.vector.tensor_mul(cn[0:64, :], ctxh[0][0:64, :], rinv[0:64, :])
            nc.vector.tensor_mul(cn[64:128, :], ctxh[1][64:128, :], rinv[64:128, :])
            cns[(qc, pair)] = cn
            if qc == QC - 1:
                # the last q-chunk's out-projection is the serial tail: split
                # it by pair (pair 0 runs during the final exp sweep, pair 1
                # accumulates via DVE right after the last ctx)
                for qsub in range(4):
                    for ec in range(2):
                        bg.append(
                            (
                                f"o3{pair}_{qsub}_{ec}",
                                make_out3_item(pair, qsub, ec, cn),
                                0.35,
                            )
                        )
            elif pair == 1:
                cn_pair = [cns.pop((qc, p)) for p in range(PAIRS)]
                for qsub in range(4):
                    for ec in range(2):
                        bg.append(
                            (
                                f"out{qc}_{qsub}_{ec}",
                                make_out_item(qc, qsub, ec, cn_pair),
                                OUT_C,
                            )
                        )

        def ctx_tick(j):
            qc, pair, kc = TICKS[j]
            if kc == 0:
                ctx_tiles[(qc, pair)] = [
                    psum.tile(
                        [128, QW], FP32, tag="ctx", bufs=2, name=f"c{qc}_{pair}_{hh}"
                    )
                    for hh in range(2)
                ]
            ctxh = ctx_tiles[(qc, pair)]
            et = etiles.pop(j)
            for hh in range(2):
                nc.tensor.matmul(
                    ctxh[hh],
                    ve[pair * 2 + hh][:, kc, :],
                    et[:, hh * QW : (hh + 1) * QW],
                    start=(kc == 0),
                    stop=(kc == KC - 1),
                )
            if kc == KC - 1:
                norm_and_queue_out(qc, pair, ctx_tiles.pop((qc, pair)))

        # ---- conductor: rate-paced issue order.  Each score frame earns
        # the PE (exp_cadence - score_cost) of budget for other work; ctx
        # trails the scores, background items (pair-1 projections, the
        # v-projection, out-projection pieces) fill the rest.  Relative
        # budgets avoid the absolute-clock modeling errors that stall the
        # ACT engine. ----
        SCORE_C, CTX_C, PROJ_C, V_C, OUT_C, SEL_C, EXP_C = (
            0.26, 0.48, 1.9, 1.1, 0.5, 0.3, 1.15,
        )

        # every projection is a deadline-forced background item; there is
        # no eager lead-in (a burst of scores clogs the in-order PE queue
        # on score-psum waits and delays everything queued behind it)
        # V leads the FIFO so the credit-paced pops spread it well before
        # the ctx sweep forces it; the k/q projections are deadline-forced
        # by their scores regardless of queue position
        bg.append(("k0_0", lambda: proj_k(0, 0), PROJ_C))
        bg.append(("q0_0", lambda: proj_q(0, 0), PROJ_C))
        for rc in range(KC):
            bg.append((f"v_{rc}", lambda rc=rc: proj_v_rc(rc), V_C))
        for sc in range(1, SC):
            bg.append((f"k0_{sc}", lambda s=sc: proj_k(0, s), PROJ_C))
            bg.append((f"q0_{sc}", lambda s=sc: proj_q(0, s), PROJ_C))
        for sc in range(SC):
            bg.append((f"k1_{sc}", lambda s=sc: proj_k(1, s), PROJ_C))
            bg.append((f"q1_{sc}", lambda s=sc: proj_q(1, s), PROJ_C))

        done = set()

        def emit_bg_one():
            label, fn, cost = bg.pop(0)
            fn()
            done.add(label)
            return cost

        def need(label):
            # pull exactly the labeled item (out of FIFO order) if pending
            if label in done:
                return 0.0
            for idx, (lb, fn, cost) in enumerate(bg):
                if lb == label:
                    bg.pop(idx)
                    fn()
                    done.add(label)
                    return cost
            raise KeyError(label)

        def emit_score(i):
            qc, pair, kc = TICKS[i]
            spent = need(f"k{pair}_{kc // 4}")
            spent += need(f"q{pair}_{qc}")
            score_tick(i)
            return spent

        def emit_ctx(j):
            qc, pair, kc = TICKS[j]
            spent = need(f"v_{kc}")
            ctx_tick(j)
            return spent + CTX_C + (SEL_C if kc == KC - 1 else 0.0)

        # Per frame: one score (ACT-paced via the score-psum semaphores),
        # then mandatory ctx at a lag that spreads the forced v-projection
        # across the front (decaying near the end so the tail is short),
        # then background items at an adaptive rate that guarantees the
        # queue drains by the last score frame.
        credit = 0.0
        sp, cp = 0, 0
        NT = len(TICKS)
        while cp < NT:
            if sp < NT:
                while sp - cp >= EXP_BUFS - 2:
                    emit_ctx(cp)
                    cp += 1
                emit_score(sp)
                sp += 1
                lag = CTX_LAG if sp < 96 else 4
                n_ctx = 2 if sp >= 96 else 1
                for _ in range(n_ctx):
                    if cp < NT and cp <= sp - lag:
                        emit_ctx(cp)
                        cp += 1
                if sp >= 8 and bg:
                    frames_left = max(1, NT - sp)
                    credit = min(
                        credit + sum(c for _, _, c in bg) / frames_left, 2.5
                    )
                    while bg and credit >= bg[0][2]:
                        credit -= bg[0][2]
                        emit_bg_one()
            elif cp < NT:
                emit_ctx(cp)
                cp += 1
        while bg:
            emit_bg_one()

    nc.finalize()
    return nc


def _numpy_reference(inputs_q, inputs_kv, Wq, bq, Wk, bk, Wv, bv, Wo, bo):
    # safety fallback (never used when inputs_kv == inputs_q, which
    # setup_inputs guarantees)
    x_q = inputs_q.astype(np.float64)
    x_kv = inputs_kv.astype(np.float64)
    q = np.einsum("bsd,dhe->bshe", x_q, Wq.astype(np.float64)) + bq
    k = np.einsum("bsd,dhe->bshe", x_kv, Wk.astype(np.float64)) + bk
    v = np.einsum("bsd,dhe->bshe", x_kv, Wv.astype(np.float64)) + bv
    q = q / np.sqrt(HD)
    s = np.einsum("bqhd,bkhd->bhqk", q, k)
    s = s - s.max(axis=-1, keepdims=True)
    e = np.exp(s)
    w = e / e.sum(axis=-1, keepdims=True)
    ctx = np.einsum("bhqk,bkhd->bqhd", w, v)
    out = np.einsum("bqhd,hde->bqe", ctx, Wo.astype(np.float64)) + bo
    return out.astype(np.float32)


def kernel(
    inputs_q, inputs_kv, Wq, bq, Wk, bk, Wv, bv, Wo, bo
):  # noqa: N803
    global LAST_EXEC_NS
    inputs_q = np.asarray(inputs_q, dtype=np.float32)
    inputs_kv = np.asarray(inputs_kv, dtype=np.float32)
    Wq = np.asarray(Wq, np.float32)
    Wk = np.asarray(Wk, np.float32)
    Wv = np.asarray(Wv, np.float32)
    Wo = np.asarray(Wo, np.float32)
    bq = np.asarray(bq, np.float32)
    bk = np.asarray(bk, np.float32)
    bv = np.asarray(bv, np.float32)
    bo = np.asarray(bo, np.float32)

    if not np.array_equal(inputs_q, inputs_kv):
        return _numpy_reference(
            inputs_q, inputs_kv, Wq, bq, Wk, bk, Wv, bv, Wo, bo
        )

    zero_bias = not (bq.any() or bk.any() or bv.any())
    key = ("prog", zero_bias)
    if key not in _PROG_CACHE:
        _PROG_CACHE[key] = _build_program(zero_bias)
    nc = _PROG_CACHE[key]

    bf16 = ml_dtypes.bfloat16

    def swz_x(xb):
        # [p, sc, dc, j] layout so each device DMA reads contiguous spans
        a = xb.T.astype(bf16).reshape(DC, 128, SC, SCW)
        return np.ascontiguousarray(a.transpose(1, 2, 0, 3)).reshape(128, -1)

    def swz_w(w):  # [D, 256] -> [p, dc, f]
        a = w.reshape(DC, 128, 256)
        return np.ascontiguousarray(a.transpose(1, 0, 2)).reshape(128, -1)

    def swz_w_pair(w):  # [D, 256] -> [p, pair, dc, f]
        a = w.reshape(DC, 128, 2, 128)
        return np.ascontiguousarray(a.transpose(1, 2, 0, 3)).reshape(128, -1)

    def swz_wo(w):  # [256, D] -> [p, pair, e]
        a = w.reshape(2, 128, D)
        return np.ascontiguousarray(a.transpose(1, 0, 2)).reshape(128, -1)

    xT_by_batch = [swz_x(inputs_kv[b]) for b in range(B)]
    in_maps = []
    for c in range(NCORES):
        b, hg = divmod(c, NCORES // B)
        hs = hg * HPC
        in_maps.append(
            {
                "xt": xT_by_batch[b],
                "wq": swz_w_pair(
                    Wq[:, hs : hs + HPC, :].reshape(D, 256).astype(bf16)
                ),
                "wk": swz_w_pair(
                    Wk[:, hs : hs + HPC, :].reshape(D, 256).astype(bf16)
                ),
                "wv": swz_w(
                    Wv[:, hs : hs + HPC, :].reshape(D, 256).astype(bf16)
                ),
                "wo": swz_wo(Wo[hs : hs + HPC].reshape(256, D)),
                "bq": np.ascontiguousarray(bq[hs : hs + HPC].reshape(2, 128).T),
                "bk": np.ascontiguousarray(bk[hs : hs + HPC].reshape(2, 128).T),
                "bv": np.ascontiguousarray(bv[hs : hs + HPC].reshape(1, 256)),
            }
        )

    trace = bool(os.environ.get("BASS_KERNEL_TRACE"))
    if trace:
        try:  # tracing needs the axon NTFF hook (test.py injects it)
            import antenv.axon_hooks  # noqa: F401
        except ImportError:
            trace = False
    res = run_bass_kernel_spmd(nc, in_maps, list(range(NCORES)), trace=trace)
    LAST_EXEC_NS = res.exec_time_ns

    out = np.empty((B, S, D), np.float32)
    for b in range(B):
        g = NCORES // B
        acc = res.results[g * b]["out_p"].copy()
        for j in range(1, g):
            acc += res.results[g * b + j]["out_p"]
        out[b] = acc + bo[None, :]
    return out


# revision 53
# speedup vs baseline: 1.0392x; 1.0392x over previous
"""Multi-head attention (B=2, S=2048, D=1024, H=16, HD=64) on 8 trn2 cores.

Sharding: core c -> (batch b = c//4, head-group hg = c%4, heads 4*hg..4*hg+3).
Each core computes its 4 heads' attention for its batch plus the partial
output projection (ctx @ Wo_slice); the host sums the 4 partials per batch
and adds bo.  setup_inputs() guarantees inputs_kv is inputs_q, which the
host verifies (falling back to a numpy reference otherwise); the host also
pre-transposes x once per batch and casts it (and Wq/Wk/Wv) to bf16, so the
device program needs no transposes at all.

The kernel is built around the observation that the softmax exp is the
hard floor: 16.8M elements/core through the ACT spline at ~1 elem/lane/
cycle @1.2GHz = ~130-147us.  Everything else (projections, score matmuls,
ctx matmuls, out-projection) is arranged so the ACT engine streams exp
continuously from ~14us after kernel start:

  1. x^T arrives in four 512-column chunks; q^T/k^T projections for head
     pair 0 chase the chunks (PE, bf16, full-rate), so scores for (qc0,
     pair0) issue per 4-kc batch as soon as each chunk is projected.
  2. scores: per kc a row-packed pair of K=64 bf16 matmuls (heads in
     disjoint PE row groups run concurrently), psum [128, 2*512].
  3. exp via ACT reading psum, scale=1/sqrt(HD) folded in, bf16 out into a
     24-deep rotating pool of per-kc tiles (decouples ACT from the ctx
     consumers by more than one full sweep).
  4. ctx^T per head (PE): lhsT = ve[:, kc, :] where ve packs v_h into the
     head's own 64 columns plus a ones-column, so one psum row is the
     softmax denominator for free.
  5. normalize: denominator rows -> sel-matmul broadcast ->
     reciprocal_approx_fast -> per-half multiply -> ctxn (fp32r).
  6. out-projection (PE, fp32r) accumulated over the two pairs, DMA'd out
     per [128, 512] chunk as produced.

A short burst of dummy sel-matmuls at t~6-10us keeps the PE HAM busy so
the real projections start at the warm 2.4GHz clock.
"""

import os
from contextlib import ExitStack

import ml_dtypes
import numpy as np

import concourse.mybir as mybir
import concourse.tile as tile
from concourse import bacc
from concourse.bass_utils import run_bass_kernel_spmd

FP32 = mybir.dt.float32
FP32R = mybir.dt.float32r
BF16 = mybir.dt.bfloat16
AF = mybir.ActivationFunctionType

B, S, D, H, HD = 2, 2048, 1024, 16, 64
NCORES = 8
HPC = 4  # heads per core
PAIRS = 2  # head pairs per core
DC = D // 128  # 8 contraction chunks
SC = 4  # s-chunks for the pipelined load/projection
SCW = S // SC  # 512
KC = S // 128  # 16 k chunks
QC = 4  # q chunks of 512
QW = 512
SCALE = 1.0 / np.sqrt(HD)
EXP_BUFS = 26  # rotating per-kc exp tiles (> ctx lag + margin)
CTX_LAG = 20  # ctx trails scores by this many ticks (spreads the v-proj)
WARMUP_MMS = 16

_PROG_CACHE = {}
LAST_EXEC_NS = None


def _build_program(zero_bias):
    nc = bacc.Bacc(None, target_bir_lowering=False, debug=False)

    # all bulk inputs are pre-swizzled on the host so every DMA below reads
    # large contiguous spans per partition (a single dma_start tops out at
    # ~70 GB/s; strided small-element reads are far worse)
    xt = nc.declare_dram_parameter("xt", [128, SC * DC * SCW], BF16, isOutput=False)
    wq = nc.declare_dram_parameter("wq", [128, DC * 256], BF16, isOutput=False)
    wk = nc.declare_dram_parameter("wk", [128, DC * 256], BF16, isOutput=False)
    wv = nc.declare_dram_parameter("wv", [128, DC * 256], BF16, isOutput=False)
    wo = nc.declare_dram_parameter("wo", [128, 2 * D], FP32R, isOutput=False)
    bq = nc.declare_dram_parameter("bq", [128, 2], FP32, isOutput=False)
    bk = nc.declare_dram_parameter("bk", [128, 2], FP32, isOutput=False)
    bv = nc.declare_dram_parameter("bv", [1, 256], FP32, isOutput=False)
    out_p = nc.declare_dram_parameter("out_p", [S, D], FP32, isOutput=True)

    # sel[k, m] broadcasts r2 row 64 to output rows 0-63 and row 0 to
    # output rows 64-127 (the two heads' denominator rows)
    sel_np = np.zeros((128, 128), np.float32)
    sel_np[64, :64] = 1.0
    sel_np[0, 64:] = 1.0
    sel_c = nc.inline_tensor(sel_np, name="sel_c")

    with ExitStack() as ctx:
        tc = ctx.enter_context(tile.TileContext(nc))

        main = ctx.enter_context(tc.tile_pool(name="main", bufs=1))
        attn = ctx.enter_context(tc.tile_pool(name="attn", bufs=1))
        psum = ctx.enter_context(tc.tile_pool(name="psum", bufs=1, space="PSUM"))

        sel = main.tile([128, 128], FP32R)
        r2 = main.tile([128, QW], FP32R)
        zsrc = main.tile([128, QW], FP32)
        bq_sb = main.tile([128, 2], FP32)
        bk_sb = main.tile([128, 2], FP32)
        bv_sb = main.tile([128, 256], FP32)
        wo_sb = main.tile([128, 2, D], FP32R)
        wq_sb = main.tile([128, 2, DC, 128], BF16)
        wk_sb = main.tile([128, 2, DC, 128], BF16)
        wv_sb = main.tile([128, DC, 256], BF16)
        xT = main.tile([128, DC, S], BF16)
        qT2 = [main.tile([128, S], BF16, name=f"qT2_{p}") for p in range(PAIRS)]
        kT2 = [main.tile([128, S], BF16, name=f"kT2_{p}") for p in range(PAIRS)]
        # ve[pair*2+hh]: per-head v with a ones column riding along so the
        # ctx matmul also produces the softmax denominator row:
        #   hh=0: cols 0-63 = v_h0, col 64 = ones, rest junk
        #   hh=1: cols 64-127 = v_h1, col 0 = ones, rest junk
        ve = [
            main.tile([128, KC, 128], BF16, name=f"ve_{i}") for i in range(2 * PAIRS)
        ]
        ones16 = main.tile([128, KC], FP32)
        warm = main.tile([128, 128], BF16)

        # ---- DMA issue: sel first (warmup needs it), weights on the
        # scalar queue (idle until the first exp), x chunks on sync ----
        # A single dma_start tops out at ~70 GB/s and each issue queue runs
        # ~2 transfers concurrently, so the first-wave tensors (x chunk 0 +
        # wk) are split in ~256KB pieces across all three queues; later
        # waves chase their compute deadlines.
        wk_v = wk.rearrange("p (h x) -> p h x", h=2)
        wq_v = wq.rearrange("p (h x) -> p h x", h=2)
        wv_v = wv.rearrange("p (h x) -> p h x", h=2)
        wo_v = wo.rearrange("p (h x) -> p h x", h=2)
        xt_q = xt.rearrange("p (c h x) -> p c h x", c=SC, h=4)

        def x_quarter(eng, sc, qi):
            eng.dma_start(
                out=xT[:, qi * 2 : (qi + 1) * 2, sc * SCW : (sc + 1) * SCW],
                in_=xt_q[:, sc, qi, :],
            )

        # bulk transfers ride the two HWDGE queues only (gpsimd SWDGE is
        # slow to generate descriptors -- tiny tensors only).  First wave:
        # x chunk 0 + the pair-0 halves of wk/wq; later x chunks and the
        # remaining weights chase their compute deadlines.
        nc.gpsimd.dma_start(out=bk_sb, in_=bk[:, :])
        nc.gpsimd.dma_start(out=bq_sb, in_=bq[:, :])
        nc.gpsimd.dma_start(out=sel, in_=sel_c[:, :])
        bv_bcast = bv[0:1, :].partition_broadcast(128)
        nc.gpsimd.dma_start(out=bv_sb, in_=bv_bcast)
        # sync interleaves the first x chunk with the pair-0 weight halves
        # (the k/q projections need both); scalar carries the other two
        # quarters of each chunk and the late weights
        x_quarter(nc.sync, 0, 0)
        nc.sync.dma_start(out=wk_sb[:, 0, :, :], in_=wk_v[:, 0, :])
        x_quarter(nc.scalar, 0, 2)
        x_quarter(nc.scalar, 0, 3)
        x_quarter(nc.sync, 0, 1)
        nc.sync.dma_start(out=wq_sb[:, 0, :, :], in_=wq_v[:, 0, :])
        for sc in range(1, SC):
            x_quarter(nc.sync, sc, 0)
            x_quarter(nc.sync, sc, 1)
            x_quarter(nc.scalar, sc, 2)
            x_quarter(nc.scalar, sc, 3)
        nc.scalar.dma_start(out=wk_sb[:, 1, :, :], in_=wk_v[:, 1, :])
        nc.scalar.dma_start(out=wq_sb[:, 1, :, :], in_=wq_v[:, 1, :])
        for hi in range(2):
            nc.scalar.dma_start(
                out=wv_sb[:, hi * 4 : (hi + 1) * 4, :], in_=wv_v[:, hi, :]
            )
        for hi in range(2):
            nc.scalar.dma_start(out=wo_sb[:, hi, :], in_=wo_v[:, hi, :])

        # ---- constants ----
        nc.vector.memset(zsrc, 0.0)
        nc.vector.tensor_copy(r2, zsrc)
        nc.vector.memset(ones16, 1.0)
        nc.vector.memset(warm, 1.0)
        for i in range(2 * PAIRS):
            col = 64 if i % 2 == 0 else 0
            nc.vector.tensor_copy(
                ve[i][:, :, col : col + 1],
                ones16.rearrange("p (a o) -> p a o", o=1),
            )

        # ---- PE warmup: dummy bf16 matmuls on a memset tile (no DMA
        # dependency) while the first x chunk is in flight, so the HAM
        # un-throttles before real work ----
        for wi in range(WARMUP_MMS):
            wps = psum.tile([128, 128], FP32, tag="work", bufs=2, name=f"warm{wi}")
            nc.tensor.matmul(wps, warm, warm, start=True, stop=True)

        # ---- projections ----
        def proj_one(nm, w_sb, b_sb, dest, pair, sc):
            pps = psum.tile(
                [128, SCW], FP32, tag="work", bufs=2, name=f"p{nm}{pair}_{sc}"
            )
            for dc in range(DC):
                nc.tensor.matmul(
                    pps,
                    w_sb[:, pair, dc, :],
                    xT[:, dc, sc * SCW : (sc + 1) * SCW],
                    start=(dc == 0),
                    stop=(dc == DC - 1),
                )
            nc.vector.tensor_scalar_add(
                dest[:, sc * SCW : (sc + 1) * SCW],
                pps,
                b_sb[:, pair : pair + 1],
            )

        def proj_k(pair, sc):
            proj_one("k", wk_sb, bk_sb, kT2[pair], pair, sc)

        def proj_q(pair, sc):
            proj_one("q", wq_sb, bq_sb, qT2[pair], pair, sc)

        def proj_v_rc(rc):
            vps = psum.tile([128, 256], FP32, tag="work", bufs=2, name=f"pv{rc}")
            for dc in range(DC):
                nc.tensor.matmul(
                    vps,
                    xT[:, dc, rc * 128 : (rc + 1) * 128],
                    wv_sb[:, dc, :],
                    start=(dc == 0),
                    stop=(dc == DC - 1),
                )
            for pair in range(PAIRS):
                nc.vector.tensor_add(
                    ve[pair * 2][:, rc, 0:64],
                    vps[:, pair * 128 : pair * 128 + 64],
                    bv_sb[:, pair * 128 : pair * 128 + 64],
                )
                nc.vector.tensor_add(
                    ve[pair * 2 + 1][:, rc, 64:128],
                    vps[:, pair * 128 + 64 : pair * 128 + 128],
                    bv_sb[:, pair * 128 + 64 : pair * 128 + 128],
                )

        # ---- attention primitives (one kc "tick" at a time) ----
        TICKS = [
            (qc, pair, kc)
            for qc in range(QC)
            for pair in range(PAIRS)
            for kc in range(KC)
        ]
        etiles = {}

        def score_tick(i):
            qc, pair, kc = TICKS[i]
            sps = psum.tile(
                [128, 1024], FP32, tag="score", bufs=2, name=f"s{qc}_{pair}_{kc}"
            )
            for hh in range(2):
                h_lo = hh * 64
                nc.tensor.matmul(
                    sps[:, hh * QW : (hh + 1) * QW],
                    kT2[pair][h_lo : h_lo + 64, kc * 128 : (kc + 1) * 128],
                    qT2[pair][h_lo : h_lo + 64, qc * QW : (qc + 1) * QW],
                    start=True,
                    stop=True,
                )
            et = attn.tile(
                [128, 1024],
                BF16,
                tag="expT",
                bufs=EXP_BUFS,
                name=f"et{qc}_{pair}_{kc}",
            )
            nc.scalar.activation(et, sps, AF.Exp, scale=float(SCALE))
            etiles[i] = et

        cns = {}
        ctx_tiles = {}
        bg = []  # deadline-checked background PE work: (label, fn, est_cost_us)

        def make_out_item(qc, qsub, ec, cn_pair):
            def fn():
                out_sb = attn.tile(
                    [128, QW], FP32, tag="outsb", bufs=4, name=f"o{qc}_{qsub}_{ec}"
                )
                r0 = qc * QW + qsub * 128
                ops = psum.tile(
                    [128, QW], FP32, tag="work", bufs=2, name=f"op{qc}{qsub}{ec}"
                )
                for pi in range(PAIRS):
                    nc.tensor.matmul(
                        ops,
                        cn_pair[pi][:, qsub * 128 : (qsub + 1) * 128],
                        wo_sb[:, pi, ec * QW : (ec + 1) * QW],
                        start=(pi == 0),
                        stop=(pi == PAIRS - 1),
                    )
                nc.vector.tensor_copy(out_sb, ops)
                nc.sync.dma_start(
                    out=out_p[r0 : r0 + 128, ec * QW : (ec + 1) * QW],
                    in_=out_sb,
                )

            return fn

        out3_acc = {}

        def make_out3_item(pair, qsub, ec, cn):
            def fn():
                ops = psum.tile(
                    [128, QW], FP32, tag="work", bufs=2, name=f"o3p{pair}{qsub}{ec}"
                )
                nc.tensor.matmul(
                    ops,
                    cn[:, qsub * 128 : (qsub + 1) * 128],
                    wo_sb[:, pair, ec * QW : (ec + 1) * QW],
                    start=True,
                    stop=True,
                )
                if pair == 0:
                    acc = attn.tile(
                        [128, QW], FP32, tag="o3acc", bufs=8, name=f"o3a{qsub}{ec}"
                    )
                    nc.vector.tensor_copy(acc, ops)
                    out3_acc[(qsub, ec)] = acc
                else:
                    acc = out3_acc.pop((qsub, ec))
                    out_sb = attn.tile(
                        [128, QW], FP32, tag="outsb", bufs=4, name=f"o3b{qsub}{ec}"
                    )
                    nc.vector.tensor_add(out_sb, ops, acc)
                    r0 = (QC - 1) * QW + qsub * 128
                    nc.sync.dma_start(
                        out=out_p[r0 : r0 + 128, ec * QW : (ec + 1) * QW],
                        in_=out_sb,
                    )

            return fn

        def norm_and_queue_out(qc, pair, ctxh):
            if qc == QC - 1 and pair == 1:
                # make sure the last q-chunk's pair-0 out-projection ran
                # before the serial tail begins
                for qs in range(4):
                    for e2 in range(2):
                        need(f"o30_{qs}_{e2}")
            # normalize: denominator rows -> r2 -> sel-matmul broadcast ->
            # approx reciprocal -> per-half mul
            nc.vector.tensor_copy(r2[64:65, :], ctxh[0][64:65, :])
            nc.vector.tensor_copy(r2[0:1, :], ctxh[1][0:1, :])
            bps = psum.tile([128, QW], FP32, tag="work", bufs=2, name=f"b{qc}_{pair}")
            nc.tensor.matmul(bps, sel, r2, start=True, stop=True)
            rinv = attn.tile([128, QW], FP32, tag="rinv", bufs=2, name=f"ri{qc}{pair}")
            nc.vector.reciprocal_approx_fast(rinv, bps)
            # the cn pool rotates 4 deep; the slot being reused is still
            # read by the victim q-chunk's out-projection matmuls, which may
            # linger in the background queue BEHIND this point of the PE
            # stream -- force them out first or the PE deadlocks on itself
            a = 2 * qc + pair
            if a >= 4:
                vq = (a - 4) // 2
                if vq < QC - 1:
                    for qs in range(4):
                        for e2 in range(2):
                            need(f"out{vq}_{qs}_{e2}")
            cn = attn.tile([128, QW], FP32R, tag="ctxn", bufs=4, name=f"cn{qc}{pair}")
            nc.vector.tensor_mul(cn[0:64, :], ctxh[0][0:64, :], rinv[0:64, :])
            nc.vector.tensor_mul(cn[64:128, :], ctxh[1][64:128, :], rinv[64:128, :])
            cns[(qc, pair)] = cn
            if qc == QC - 1:
                # the last q-chunk's out-projection is the serial tail: split
                # it by pair (pair 0 runs during the final exp sweep, pair 1
                # accumulates via DVE right after the last ctx)
                for qsub in range(4):
                    for ec in range(2):
                        bg.append(
                            (
                                f"o3{pair}_{qsub}_{ec}",
                                make_out3_item(pair, qsub, ec, cn),
                                0.35,
                            )
                        )
            elif pair == 1:
                cn_pair = [cns.pop((qc, p)) for p in range(PAIRS)]
                for qsub in range(4):
                    for ec in range(2):
                        bg.append(
                            (
                                f"out{qc}_{qsub}_{ec}",
                                make_out_item(qc, qsub, ec, cn_pair),
                                OUT_C,
                            )
                        )

        def ctx_tick(j):
            qc, pair, kc = TICKS[j]
            if kc == 0:
                ctx_tiles[(qc, pair)] = [
                    psum.tile(
                        [128, QW], FP32, tag="ctx", bufs=2, name=f"c{qc}_{pair}_{hh}"
                    )
                    for hh in range(2)
                ]
            ctxh = ctx_tiles[(qc, pair)]
            et = etiles.pop(j)
            for hh in range(2):
                nc.tensor.matmul(
                    ctxh[hh],
                    ve[pair * 2 + hh][:, kc, :],
                    et[:, hh * QW : (hh + 1) * QW],
                    start=(kc == 0),
                    stop=(kc == KC - 1),
                )
            if kc == KC - 1:
                norm_and_queue_out(qc, pair, ctx_tiles.pop((qc, pair)))

        # ---- conductor: rate-paced issue order.  Each score frame earns
        # the PE (exp_cadence - score_cost) of budget for other work; ctx
        # trails the scores, background items (pair-1 projections, the
        # v-projection, out-projection pieces) fill the rest.  Relative
        # budgets avoid the absolute-clock modeling errors that stall the
        # ACT engine. ----
        SCORE_C, CTX_C, PROJ_C, V_C, OUT_C, SEL_C, EXP_C = (
            0.26, 0.48, 1.9, 1.1, 0.5, 0.3, 1.15,
        )

        # every projection is a deadline-forced background item; there is
        # no eager lead-in (a burst of scores clogs the in-order PE queue
        # on score-psum waits and delays everything queued behind it)
        # every projection is a deadline-forced background item; there is
        # no eager lead-in (a burst of scores clogs the in-order PE queue
        # on score-psum waits and delays everything queued behind it)
        for pair in range(PAIRS):
            for sc in range(SC):
                bg.append(
                    (f"k{pair}_{sc}", lambda p=pair, s=sc: proj_k(p, s), PROJ_C)
                )
                if sc == 0 or pair == 0:
                    bg.append(
                        (f"q{pair}_{sc}", lambda p=pair, s=sc: proj_q(p, s), PROJ_C)
                    )
        for rc in range(KC):
            bg.append((f"v_{rc}", lambda rc=rc: proj_v_rc(rc), V_C))
        for sc in range(1, SC):
            bg.append((f"q1_{sc}", lambda sc=sc: proj_q(1, sc), PROJ_C))

        done = set()

        def emit_bg_one():
            label, fn, cost = bg.pop(0)
            fn()
            done.add(label)
            return cost

        def need(label):
            # pull exactly the labeled item (out of FIFO order) if pending
            if label in done:
                return 0.0
            for idx, (lb, fn, cost) in enumerate(bg):
                if lb == label:
                    bg.pop(idx)
                    fn()
                    done.add(label)
                    return cost
            raise KeyError(label)

        def emit_score(i):
            qc, pair, kc = TICKS[i]
            spent = need(f"k{pair}_{kc // 4}")
            spent += need(f"q{pair}_{qc}")
            score_tick(i)
            return spent

        def emit_ctx(j):
            qc, pair, kc = TICKS[j]
            spent = need(f"v_{kc}")
            ctx_tick(j)
            return spent + CTX_C + (SEL_C if kc == KC - 1 else 0.0)

        # Per frame: one score (ACT-paced via the score-psum semaphores),
        # then mandatory ctx at a lag that spreads the forced v-projection
        # across the front (decaying near the end so the tail is short),
        # then background items at an adaptive rate that guarantees the
        # queue drains by the last score frame.
        credit = 0.0
        sp, cp = 0, 0
        NT = len(TICKS)
        while cp < NT:
            if sp < NT:
                while sp - cp >= EXP_BUFS - 2:
                    emit_ctx(cp)
                    cp += 1
                emit_score(sp)
                sp += 1
                lag = CTX_LAG if sp < 96 else 4
                n_ctx = 2 if sp >= 96 else 1
                for _ in range(n_ctx):
                    if cp < NT and cp <= sp - lag:
                        emit_ctx(cp)
                        cp += 1
                if sp >= 8 and bg:
                    frames_left = max(1, NT - sp)
                    credit = min(
                        credit + sum(c for _, _, c in bg) / frames_left, 2.5
                    )
                    while bg and credit >= bg[0][2]:
                        credit -= bg[0][2]
                        emit_bg_one()
            elif cp < NT:
                emit_ctx(cp)
                cp += 1
        while bg:
            emit_bg_one()

    nc.finalize()
    return nc


def _numpy_reference(inputs_q, inputs_kv, Wq, bq, Wk, bk, Wv, bv, Wo, bo):
    # safety fallback (never used when inputs_kv == inputs_q, which
    # setup_inputs guarantees)
    x_q = inputs_q.astype(np.float64)
    x_kv = inputs_kv.astype(np.float64)
    q = np.einsum("bsd,dhe->bshe", x_q, Wq.astype(np.float64)) + bq
    k = np.einsum("bsd,dhe->bshe", x_kv, Wk.astype(np.float64)) + bk
    v = np.einsum("bsd,dhe->bshe", x_kv, Wv.astype(np.float64)) + bv
    q = q / np.sqrt(HD)
    s = np.einsum("bqhd,bkhd->bhqk", q, k)
    s = s - s.max(axis=-1, keepdims=True)
    e = np.exp(s)
    w = e / e.sum(axis=-1, keepdims=True)
    ctx = np.einsum("bhqk,bkhd->bqhd", w, v)
    out = np.einsum("bqhd,hde->bqe", ctx, Wo.astype(np.float64)) + bo
    return out.astype(np.float32)


def kernel(
    inputs_q, inputs_kv, Wq, bq, Wk, bk, Wv, bv, Wo, bo
):  # noqa: N803
    global LAST_EXEC_NS
    inputs_q = np.asarray(inputs_q, dtype=np.float32)
    inputs_kv = np.asarray(inputs_kv, dtype=np.float32)
    Wq = np.asarray(Wq, np.float32)
    Wk = np.asarray(Wk, np.float32)
    Wv = np.asarray(Wv, np.float32)
    Wo = np.asarray(Wo, np.float32)
    bq = np.asarray(bq, np.float32)
    bk = np.asarray(bk, np.float32)
    bv = np.asarray(bv, np.float32)
    bo = np.asarray(bo, np.float32)

    if not np.array_equal(inputs_q, inputs_kv):
        return _numpy_reference(
            inputs_q, inputs_kv, Wq, bq, Wk, bk, Wv, bv, Wo, bo
        )

    zero_bias = not (bq.any() or bk.any() or bv.any())
    key = ("prog", zero_bias)
    if key not in _PROG_CACHE:
        _PROG_CACHE[key] = _build_program(zero_bias)
    nc = _PROG_CACHE[key]

    bf16 = ml_dtypes.bfloat16

    def swz_x(xb):
        # [p, sc, dc, j] layout so each device DMA reads contiguous spans
        a = xb.T.astype(bf16).reshape(DC, 128, SC, SCW)
        return np.ascontiguousarray(a.transpose(1, 2, 0, 3)).reshape(128, -1)

    def swz_w(w):  # [D, 256] -> [p, dc, f]
        a = w.reshape(DC, 128, 256)
        return np.ascontiguousarray(a.transpose(1, 0, 2)).reshape(128, -1)

    def swz_w_pair(w):  # [D, 256] -> [p, pair, dc, f]
        a = w.reshape(DC, 128, 2, 128)
        return np.ascontiguousarray(a.transpose(1, 2, 0, 3)).reshape(128, -1)

    def swz_wo(w):  # [256, D] -> [p, pair, e]
        a = w.reshape(2, 128, D)
        return np.ascontiguousarray(a.transpose(1, 0, 2)).reshape(128, -1)

    xT_by_batch = [swz_x(inputs_kv[b]) for b in range(B)]
    in_maps = []
    for c in range(NCORES):
        b, hg = divmod(c, NCORES // B)
        hs = hg * HPC
        in_maps.append(
            {
                "xt": xT_by_batch[b],
                "wq": swz_w_pair(
                    Wq[:, hs : hs + HPC, :].reshape(D, 256).astype(bf16)
                ),
                "wk": swz_w_pair(
                    Wk[:, hs : hs + HPC, :].reshape(D, 256).astype(bf16)
                ),
                "wv": swz_w(
                    Wv[:, hs : hs + HPC, :].reshape(D, 256).astype(bf16)
                ),
                "wo": swz_wo(Wo[hs : hs + HPC].reshape(256, D)),
                "bq": np.ascontiguousarray(bq[hs : hs + HPC].reshape(2, 128).T),
                "bk": np.ascontiguousarray(bk[hs : hs + HPC].reshape(2, 128).T),
                "bv": np.ascontiguousarray(bv[hs : hs + HPC].reshape(1, 256)),
            }
        )

    trace = bool(os.environ.get("BASS_KERNEL_TRACE"))
    if trace:
        try:  # tracing needs the axon NTFF hook (test.py injects it)
            import antenv.axon_hooks  # noqa: F401
        except ImportError:
            trace = False
    res = run_bass_kernel_spmd(nc, in_maps, list(range(NCORES)), trace=trace)
    LAST_EXEC_NS = res.exec_time_ns

    out = np.empty((B, S, D), np.float32)
    for b in range(B):
        g = NCORES // B
        acc = res.results[g * b]["out_p"].copy()
        for j in range(1, g):
            acc += res.results[g * b + j]["out_p"]
        out[b] = acc + bo[None, :]
    return out
